# revision 16
# baseline (speedup 1.0000x reference)
"""NonLocalBlock (single-head attention, N=HW=4096, d=128) on 8 trn2 cores.

Sharding: data-parallel over batch (B=8) — one batch element per NeuronCore.
Per core, the whole block runs out of SBUF:

  xf (256, 4096) -> theta_T = wt@xf + bt      (128, N)   [PE + bias on copy]
                    phi     = wp@xf           (128, N)   [PE; bp dropped]
                    gT      = wg@xf           (128, N)   [PE]
                    g0      = gT^T chunks     (N, 128)   [xbar DMA transpose]
  S^T[m, n] = sum_i phi[i,m] * theta_T[i,n]   (keys m on partitions)
  expS = exp(S^T - 40)                         [ACT, some chunks DVE bit-trick]
  sums[n] = sum_m expS[m, n]                   [DVE bf16 partial adds + PE fold]
  yT[o, n] = (sum_m g0[m,o] expS[m,n]) / sums[n]
  out = wW @ yT + (wW@bg + bW) + xf

Numerics:
 - phi's bias bp only adds a per-query constant to S -> softmax-invariant,
   dropped entirely.
 - No per-row max: scores ~N(0,128), exp(S-40) stays in range (see analysis).
 - theta/phi stored fp16: stationary operand gets FWL (2x faster weight
   load); fp16 keeps the absolute score error ~3e-3 (negligible through exp).
 - DVE_EXP_MCS key-chunks per quarter compute exp on the Vector engine via
   the Schraudolph bit trick: bf16_bits(e^x) ~= rint(x*128*log2e + 16256) as
   one tensor_scalar (fp32 PSUM -> uint16, HW round-to-nearest + saturate),
   bitcast to bf16.  ~3.3% max rel err on those chunks' weights; offloads
   the otherwise-bottleneck ACT exp stream.

Schedule: flat 128-step loop (4 query-quarters x 32 key-chunks).  AV matmuls
run 2 chunks behind S for elasticity; each quarter's tail (fold / reciprocal
/ normalize / Wy / +x / store) is emitted inside the next quarter's first
steps in 512-column blocks so no engine drains; projections stream behind
the x DMA; dummy PE warmup keeps HAM from throttling the prologue.
"""

import numpy as np
from contextlib import ExitStack

import concourse.bass as bass
import concourse.mybir as mybir
import concourse.tile as tile
from concourse import bacc

P = 128          # partitions / inter channels
C = 256          # input channels
F32 = mybir.dt.float32
F32R = mybir.dt.float32r
FP16 = mybir.dt.float16
U16 = mybir.dt.uint16
BF16 = mybir.dt.bfloat16
AF = mybir.ActivationFunctionType
ALU = mybir.AluOpType
CSHIFT = 40.0    # global score shift before exp (softmax-invariant)

LOG2E = 1.4426950408889634
SCHR_MUL = float(np.float32(128 * LOG2E))
SCHR_ADD = float(np.float32(16256 - CSHIFT * 128 * LOG2E - 5.61))

B_FULL = 8
H_FULL = 64
W_FULL = 64
N_FULL = H_FULL * W_FULL

NQ = 1024                     # query-quarter width
# key-chunks (of 32 per quarter) whose exp runs on DVE instead of ACT
DVE_EXP_MCS = (4, 11, 17, 23, 27)
WARMUP_MMS = 38               # keep PE ticking until the first x block lands
G_VIA_TRANSPOSE = False       # gT + xbar transpose vs per-chunk matmuls


def build_nc(N=N_FULL):
    MC = N // P                   # 32 key chunks
    NQn = N // NQ                 # 4 query quarters
    NB = NQ // 512                # 2 512-wide blocks per quarter

    nc = bacc.Bacc("TRN2", target_bir_lowering=False, debug=False)

    x_d = nc.dram_tensor("x", [C, N], F32R, kind="ExternalInput").ap()
    wtT_d = nc.dram_tensor("wtT", [P, 2 * P], F32R, kind="ExternalInput").ap()
    wpT_d = nc.dram_tensor("wpT", [P, 2 * P], F32R, kind="ExternalInput").ap()
    wgT_d = nc.dram_tensor("wgT", [P, 2 * P], F32R, kind="ExternalInput").ap()
    wWT_d = nc.dram_tensor("wWT", [P, C], F32R, kind="ExternalInput").ap()
    bt_d = nc.dram_tensor("bt", [P, 1], F32, kind="ExternalInput").ap()
    bWp_d = nc.dram_tensor("bWp", [P, 2], F32, kind="ExternalInput").ap()
    out_d = nc.dram_tensor("out", [C, N], F32, kind="ExternalOutput").ap()

    x_v = x_d.rearrange("(k p) n -> k p n", p=P)
    out_v = out_d.rearrange("(k p) n -> k p n", p=P)

    with tile.TileContext(nc) as tc, ExitStack() as ctx:
        const = ctx.enter_context(tc.tile_pool(name="const", bufs=1))
        big = ctx.enter_context(tc.tile_pool(name="big", bufs=1))
        work = ctx.enter_context(tc.tile_pool(name="work", bufs=3))
        ps = ctx.enter_context(tc.tile_pool(name="ps", bufs=3, space="PSUM"))
        psy = ctx.enter_context(tc.tile_pool(name="psy", bufs=1, space="PSUM"))

        # ---- constants ----
        wtT_sb = const.tile([P, 2, P], F32R, name="wtT_sb")
        wpT_sb = const.tile([P, 2, P], F32R, name="wpT_sb")
        wgT_sb = const.tile([P, 2, P], F32R, name="wgT_sb")
        wWT_sb = const.tile([P, C], F32R, name="wWT_sb")
        bt_sb = const.tile([P, 1], F32, name="bt_sb")
        bWp_sb = const.tile([P, 2], F32, name="bWp_sb")
        ones_sb = const.tile([P, P], BF16, name="ones_sb")
        cshift_sb = const.tile([P, 1], F32, name="cshift_sb")
        nc.vector.memset(cshift_sb[:], -CSHIFT)
        nc.vector.memset(ones_sb[:], 1.0)

        x_sb = big.tile([P, 2, N], F32R, name="x_sb")

        def x_dma(b):
            for k in range(2):
                nc.sync.dma_start(
                    x_sb[:, k, b * 512:(b + 1) * 512],
                    x_v[k, :, b * 512:(b + 1) * 512],
                )

        # x triggers on the SP queue; weights go out on the Activation
        # engine's HWDGE queue in parallel (each dma_start trigger costs
        # ~0.6us of queue time, so serializing all of them delays theta)
        x_dma(0)
        x_dma(1)
        nc.scalar.dma_start(wtT_sb[:], wtT_d.rearrange("p (k i) -> p k i", k=2))
        nc.scalar.dma_start(wpT_sb[:], wpT_d.rearrange("p (k i) -> p k i", k=2))
        nc.scalar.dma_start(wgT_sb[:], wgT_d.rearrange("p (k i) -> p k i", k=2))
        nc.scalar.dma_start(wWT_sb[:], wWT_d)
        nc.scalar.dma_start(bt_sb[:], bt_d)
        nc.scalar.dma_start(bWp_sb[:], bWp_d)
        x_dma(2)

        # ---- PE warmup: keep the HAM activity window busy while the x DMA
        # streams in, so real matmuls start at 2.4 GHz instead of 1.2.
        warm_ps = ps.tile([P, NQ], F32, tag="s", name="warm_ps")
        for _ in range(WARMUP_MMS):
            nc.tensor.matmul(warm_ps[:, 0:P], ones_sb[:], ones_sb[:],
                             start=True, stop=True, skip_group_check=True)

        th_sb = big.tile([P, N], FP16, name="th_sb")   # theta^T (i, n)
        ph_sb = big.tile([P, N], FP16, name="ph_sb")   # phi (i, m)
        gT_sb = big.tile([P, N], BF16, name="gT_sb")   # g0^T (o, m)
        g_sb = big.tile([P, MC, P], BF16, name="g_sb")  # g0 (m_in, chunk, o)

        def proj_block(b, wT, dst, bias):
            sl = slice(b * 512, (b + 1) * 512)
            p_ps = ps.tile([P, NQ], F32, tag="s", name="p_ps")
            nc.tensor.matmul(p_ps[:, 0:512], wT[:, 0], x_sb[:, 0, sl],
                             start=True, stop=False)
            nc.tensor.matmul(p_ps[:, 0:512], wT[:, 1], x_sb[:, 1, sl],
                             start=False, stop=True)
            if bias is None:
                nc.scalar.copy(dst[:, sl], p_ps[:, 0:512])
            else:
                nc.scalar.activation(dst[:, sl], p_ps[:, 0:512], AF.Identity,
                                     bias=bias)

        # per-quarter state carried across the flat loop
        state = {}

        def start_quarter(q):
            state[q] = {
                "y": psy.tile([P, NQ], F32, tag="y", name=f"y{q}_ps"),
                "acc": [None],
                "exp": [None] * MC,
            }

        def emit_S(q, mc):
            st = state[q]
            msl = slice(mc * P, (mc + 1) * P)
            s_ps = ps.tile([P, NQ], F32, tag="s", name="s_ps")
            for b in range(NB):
                nc.tensor.matmul(
                    s_ps[:, b * 512:(b + 1) * 512], ph_sb[:, msl],
                    th_sb[:, q * NQ + b * 512: q * NQ + (b + 1) * 512],
                    start=True, stop=True)
            exp_t = work.tile([P, NQ], BF16, tag="exp", bufs=7, name="exp_sb")
            if mc in DVE_EXP_MCS:
                nc.vector.tensor_scalar(
                    exp_t[:].bitcast(U16), s_ps[:], SCHR_MUL, SCHR_ADD,
                    ALU.mult, ALU.add)
            else:
                nc.scalar.activation(exp_t[:], s_ps[:], AF.Exp,
                                     bias=cshift_sb[:, 0:1])
            st["exp"][mc] = exp_t

        def emit_AV(q, mc):
            st = state[q]
            exp_t = st["exp"][mc]
            for b in range(NB):
                bsl = slice(b * 512, (b + 1) * 512)
                nc.tensor.matmul(
                    st["y"][:, bsl], g_sb[:, mc], exp_t[:, bsl],
                    start=(mc == 0), stop=(mc == MC - 1),
                    skip_group_check=True)
            if st["acc"][0] is None:
                st["acc"][0] = work.tile([P, NQ], BF16, tag="acc0",
                                         bufs=2, name="acc0_sb")
                nc.vector.tensor_copy(st["acc"][0][:], exp_t[:])
            else:
                nc.vector.tensor_add(st["acc"][0][:], st["acc"][0][:],
                                     exp_t[:])
            st["exp"][mc] = None

        def finish_quarter(q):
            # flush the AV skew, then fold the denominator partials
            emit_AV(q, MC - 3)
            emit_AV(q, MC - 2)
            emit_AV(q, MC - 1)
            st = state[q]
            sumt = ps.tile([P, NQ], F32, tag="s", name="sumt_ps")
            for b in range(NB):
                bsl = slice(b * 512, (b + 1) * 512)
                nc.tensor.matmul(sumt[:, bsl], ones_sb[:],
                                 st["acc"][0][:, bsl],
                                 start=True, stop=True,
                                 skip_group_check=True)
            st["sumt"] = sumt

        def emit_norm(q):
            """1/sums and normalized y^T, both 512-col blocks (DVE)."""
            st = state[q]
            st["recip"] = work.tile([P, NQ], F32, tag="recip", bufs=2,
                                    name="recip_sb")
            st["yt"] = work.tile([P, NQ], F32R, tag="yt", bufs=2,
                                 name="yt_sb")
            for b in range(NB):
                bsl = slice(b * 512, (b + 1) * 512)
                nc.vector.reciprocal_approx_fast(st["recip"][:, bsl],
                                                 st["sumt"][:, bsl])
                nc.vector.tensor_mul(st["yt"][:, bsl], st["y"][:, bsl],
                                     st["recip"][:, bsl])

        def emit_wy(q):
            st = state[q]
            st["wy"] = [ps.tile([P, NQ], F32, tag="s", name=f"wy{h}_ps")
                        for h in range(2)]
            for b in range(NB):
                bsl = slice(b * 512, (b + 1) * 512)
                for h in range(2):
                    nc.tensor.matmul(st["wy"][h][:, bsl],
                                     wWT_sb[:, h * P:(h + 1) * P],
                                     st["yt"][:, bsl], start=True, stop=True)

        def emit_o(q, add_engine):
            st = state[q]
            o = [work.tile([P, NQ], F32, tag=f"o{h}", bufs=2,
                           name=f"o{h}_sb") for h in range(2)]
            for b in range(NB):
                bsl = slice(b * 512, (b + 1) * 512)
                csl = slice(q * NQ + b * 512, q * NQ + (b + 1) * 512)
                for h in range(2):
                    nc.scalar.activation(o[h][:, bsl], st["wy"][h][:, bsl],
                                         AF.Identity, bias=bWp_sb[:, h:h + 1])
                    add_engine.tensor_add(o[h][:, bsl], o[h][:, bsl],
                                          x_sb[:, h, csl])
                    nc.sync.dma_start(out_v[h, :, csl], o[h][:, bsl])

        # ---- emission ----
        th_ready = 0

        def emit_th(b):
            proj_block(b, wtT_sb, th_sb, bt_sb[:, 0:1])

        emit_th(0)
        emit_th(1)

        for t in range(NQn * MC):
            q, mc = divmod(t, MC)
            if mc == 0:
                start_quarter(q)
            if q == 0 and mc % 4 == 0:
                b = mc // 4
                if b + 3 <= 7:
                    x_dma(b + 3)
                proj_block(b, wpT_sb, ph_sb, None)     # phi keys block
                if G_VIA_TRANSPOSE:
                    proj_block(b, wgT_sb, gT_sb, None)  # gT keys block
                    for m2 in range(4 * b, 4 * b + 4):  # g chunks via xbar
                        nc.sync.dma_start_transpose(
                            g_sb[:, m2], gT_sb[:, m2 * P:(m2 + 1) * P])
                else:
                    for m2 in range(4 * b, 4 * b + 4):
                        g_ps = ps.tile([P, NQ], F32, tag="s", name="g_ps")
                        msl = slice(m2 * P, (m2 + 1) * P)
                        nc.tensor.matmul(g_ps[:, 0:P], x_sb[:, 0, msl],
                                         wgT_sb[:, 0], start=True, stop=False)
                        nc.tensor.matmul(g_ps[:, 0:P], x_sb[:, 1, msl],
                                         wgT_sb[:, 1], start=False, stop=True)
                        if m2 % 2 == 0:
                            nc.vector.tensor_copy(g_sb[:, m2], g_ps[:, 0:P])
                        else:
                            nc.scalar.copy(g_sb[:, m2], g_ps[:, 0:P])
            if q < NQn - 1 and mc in (8, 16):
                emit_th(2 * (q + 1) + (mc == 16))
            # previous quarter's pipelined tail, emitted BEFORE this step's
            # psum allocations so pool recycling can't outrun its readers
            if q > 0:
                if mc == 2:
                    emit_norm(q - 1)
                elif mc == 3:
                    emit_wy(q - 1)
                elif mc == 4:
                    emit_o(q - 1, nc.vector)
                    del state[q - 1]
            emit_S(q, mc)
            if q > 0 and mc == 1:
                finish_quarter(q - 1)
            # AV skew: 3 behind normally; first AVs of q>0 wait for the
            # previous quarter's y_ps to be fully consumed (psy bufs=1)
            if q == 0:
                if mc >= 3:
                    emit_AV(q, mc - 3)
            else:
                if mc in (3, 4, 5):
                    emit_AV(q, mc - 3)
                elif mc >= 6:
                    emit_AV(q, mc - 3)

        # final quarter: block-pipelined tail so the first 512 columns hit
        # the output DMA while the second block is still normalizing
        qf = NQn - 1
        finish_quarter(qf)
        st = state[qf]
        st["recip"] = work.tile([P, NQ], F32, tag="recip", bufs=2,
                                name="recip_sb")
        st["yt"] = work.tile([P, NQ], F32R, tag="yt", bufs=2, name="yt_sb")
        st["wy"] = [ps.tile([P, NQ], F32, tag="s", name=f"wyf{h}_ps")
                    for h in range(2)]
        of = [work.tile([P, NQ], F32, tag=f"o{h}", bufs=2, name=f"of{h}_sb")
              for h in range(2)]
        for b in range(NB):
            bsl = slice(b * 512, (b + 1) * 512)
            csl = slice(qf * NQ + b * 512, qf * NQ + (b + 1) * 512)
            nc.vector.reciprocal_approx_fast(st["recip"][:, bsl],
                                             st["sumt"][:, bsl])
            nc.vector.tensor_mul(st["yt"][:, bsl], st["y"][:, bsl],
                                 st["recip"][:, bsl])
            for h in range(2):
                nc.tensor.matmul(st["wy"][h][:, bsl],
                                 wWT_sb[:, h * P:(h + 1) * P],
                                 st["yt"][:, bsl], start=True, stop=True)
            for h in range(2):
                nc.scalar.activation(of[h][:, bsl], st["wy"][h][:, bsl],
                                     AF.Identity, bias=bWp_sb[:, h:h + 1])
                nc.vector.tensor_add(of[h][:, bsl], of[h][:, bsl],
                                     x_sb[:, h, csl])
                nc.sync.dma_start(out_v[h, :, csl], of[h][:, bsl])

    nc.compile()
    return nc


_CACHE = {}


def _built(key=(N_FULL,)):
    if key not in _CACHE:
        _CACHE[key] = build_nc(*key)
    return _CACHE[key]


def make_in_maps(x, wg, bg, wt, bt, wp, bp, wW, bW):
    """Host-side prep: per-core input dicts (core b <- batch b)."""
    x = np.asarray(x, np.float32)
    B, C_, H, W = x.shape
    N = H * W
    xf = np.ascontiguousarray(x.reshape(B, C_, N))
    wg, bg, wt, bt, wp, bp, wW, bW = [
        np.asarray(a, np.float32) for a in (wg, bg, wt, bt, wp, bp, wW, bW)]

    def pack(w):  # (128, C) conv weight -> partition-major lhsT chunks
        return np.ascontiguousarray(
            w.T.reshape(2, P, P).transpose(1, 0, 2).reshape(P, 2 * P))

    wtT, wpT, wgT = pack(wt), pack(wp), pack(wg)
    wWT = np.ascontiguousarray(wW.T)                       # (128, 256)
    bWp = (wW @ bg + bW).astype(np.float32)                # fold bg into bW
    bWp = np.ascontiguousarray(bWp.reshape(2, P).T)        # (128, 2)
    shared = {
        "wtT": wtT, "wpT": wpT, "wgT": wgT, "wWT": wWT,
        "bt": bt.reshape(P, 1).copy(), "bWp": bWp,
    }
    return [{"x": np.ascontiguousarray(xf[b]), **shared} for b in range(B)]


def kernel(x, wg, bg, wt, bt, wp, bp, wW, bW):
    from concourse.bass_utils import run_bass_kernel_spmd

    B, C_, H, W = np.asarray(x).shape
    in_maps = make_in_maps(x, wg, bg, wt, bt, wp, bp, wW, bW)
    nc = _built()
    res = run_bass_kernel_spmd(nc, in_maps, core_ids=list(range(B)))
    out = np.stack([res.results[b]["out"] for b in range(B)])
    return out.reshape(B, C_, H, W).astype(np.float32)


# revision 17
# speedup vs baseline: 1.0631x; 1.0631x over previous
"""NonLocalBlock (single-head attention, N=HW=4096, d=128) on 8 trn2 cores.

Sharding: data-parallel over batch (B=8) — one batch element per NeuronCore.
Per core, the whole block runs out of SBUF:

  xf (256, 4096) -> theta_T = wt@xf + bt      (128, N)   [PE + bias on copy]
                    phi     = wp@xf           (128, N)   [PE; bp dropped]
                    gT      = wg@xf           (128, N)   [PE]
                    g0      = gT^T chunks     (N, 128)   [xbar DMA transpose]
  S^T[m, n] = sum_i phi[i,m] * theta_T[i,n]   (keys m on partitions)
  expS = exp(S^T - 40)                         [ACT, some chunks DVE bit-trick]
  sums[n] = sum_m expS[m, n]                   [DVE bf16 partial adds + PE fold]
  yT[o, n] = (sum_m g0[m,o] expS[m,n]) / sums[n]
  out = wW @ yT + (wW@bg + bW) + xf

Numerics:
 - phi's bias bp only adds a per-query constant to S -> softmax-invariant,
   dropped entirely.
 - No per-row max: scores ~N(0,128), exp(S-40) stays in range (see analysis).
 - theta/phi stored fp16: stationary operand gets FWL (2x faster weight
   load); fp16 keeps the absolute score error ~3e-3 (negligible through exp).
 - DVE_EXP_MCS key-chunks per quarter compute exp on the Vector engine via
   the Schraudolph bit trick: bf16_bits(e^x) ~= rint(x*128*log2e + 16256) as
   one tensor_scalar (fp32 PSUM -> uint16, HW round-to-nearest + saturate),
   bitcast to bf16.  ~3.3% max rel err on those chunks' weights; offloads
   the otherwise-bottleneck ACT exp stream.

Schedule: flat 128-step loop (4 query-quarters x 32 key-chunks).  AV matmuls
run 2 chunks behind S for elasticity; each quarter's tail (fold / reciprocal
/ normalize / Wy / +x / store) is emitted inside the next quarter's first
steps in 512-column blocks so no engine drains; projections stream behind
the x DMA; dummy PE warmup keeps HAM from throttling the prologue.
"""

import numpy as np
from contextlib import ExitStack

import concourse.bass as bass
import concourse.mybir as mybir
import concourse.tile as tile
from concourse import bacc

P = 128          # partitions / inter channels
C = 256          # input channels
F32 = mybir.dt.float32
F32R = mybir.dt.float32r
FP16 = mybir.dt.float16
U16 = mybir.dt.uint16
BF16 = mybir.dt.bfloat16
AF = mybir.ActivationFunctionType
ALU = mybir.AluOpType
CSHIFT = 40.0    # global score shift before exp (softmax-invariant)

LOG2E = 1.4426950408889634
SCHR_MUL = float(np.float32(128 * LOG2E))
SCHR_ADD = float(np.float32(16256 - CSHIFT * 128 * LOG2E - 5.61))

B_FULL = 8
H_FULL = 64
W_FULL = 64
N_FULL = H_FULL * W_FULL

NQ = 1024                     # query-quarter width
# key-chunks (of 32 per quarter) whose exp runs on DVE instead of ACT
DVE_EXP_MCS = (4, 11, 17, 23, 27)
WARMUP_MMS = 38               # keep PE ticking until the first x block lands
G_VIA_TRANSPOSE = False       # gT + xbar transpose vs per-chunk matmuls


def build_nc(N=N_FULL):
    MC = N // P                   # 32 key chunks
    NQn = N // NQ                 # 4 query quarters
    NB = NQ // 512                # 2 512-wide blocks per quarter

    nc = bacc.Bacc("TRN2", target_bir_lowering=False, debug=False)

    x_d = nc.dram_tensor("x", [C, N], F32R, kind="ExternalInput").ap()
    wtT_d = nc.dram_tensor("wtT", [P, 2 * P], F32R, kind="ExternalInput").ap()
    wpT_d = nc.dram_tensor("wpT", [P, 2 * P], F32R, kind="ExternalInput").ap()
    wgT_d = nc.dram_tensor("wgT", [P, 2 * P], F32R, kind="ExternalInput").ap()
    wWT_d = nc.dram_tensor("wWT", [P, C], F32R, kind="ExternalInput").ap()
    bt_d = nc.dram_tensor("bt", [P, 1], F32, kind="ExternalInput").ap()
    bWp_d = nc.dram_tensor("bWp", [P, 2], F32, kind="ExternalInput").ap()
    out_d = nc.dram_tensor("out", [C, N], F32, kind="ExternalOutput").ap()

    x_v = x_d.rearrange("(k p) n -> k p n", p=P)
    out_v = out_d.rearrange("(k p) n -> k p n", p=P)

    with tile.TileContext(nc) as tc, ExitStack() as ctx:
        const = ctx.enter_context(tc.tile_pool(name="const", bufs=1))
        big = ctx.enter_context(tc.tile_pool(name="big", bufs=1))
        work = ctx.enter_context(tc.tile_pool(name="work", bufs=3))
        ps = ctx.enter_context(tc.tile_pool(name="ps", bufs=3, space="PSUM"))
        psy = ctx.enter_context(tc.tile_pool(name="psy", bufs=1, space="PSUM"))

        # ---- constants ----
        wtT_sb = const.tile([P, 2, P], F32R, name="wtT_sb")
        wpT_sb = const.tile([P, 2, P], F32R, name="wpT_sb")
        wgT_sb = const.tile([P, 2, P], F32R, name="wgT_sb")
        wWT_sb = const.tile([P, C], F32R, name="wWT_sb")
        bt_sb = const.tile([P, 1], F32, name="bt_sb")
        bWp_sb = const.tile([P, 2], F32, name="bWp_sb")
        ones_sb = const.tile([P, P], BF16, name="ones_sb")
        cshift_sb = const.tile([P, 1], F32, name="cshift_sb")
        nc.vector.memset(cshift_sb[:], -CSHIFT)
        nc.vector.memset(ones_sb[:], 1.0)

        x_sb = big.tile([P, 2, N], F32R, name="x_sb")

        def x_dma(b):
            for k in range(2):
                nc.sync.dma_start(
                    x_sb[:, k, b * 512:(b + 1) * 512],
                    x_v[k, :, b * 512:(b + 1) * 512],
                )

        # x triggers on the SP queue; weights go out on the Activation
        # engine's HWDGE queue in parallel (each dma_start trigger costs
        # ~0.6us of queue time, so serializing all of them delays theta)
        x_dma(0)
        x_dma(1)
        nc.scalar.dma_start(wtT_sb[:], wtT_d.rearrange("p (k i) -> p k i", k=2))
        nc.scalar.dma_start(wpT_sb[:], wpT_d.rearrange("p (k i) -> p k i", k=2))
        nc.scalar.dma_start(wgT_sb[:], wgT_d.rearrange("p (k i) -> p k i", k=2))
        nc.scalar.dma_start(wWT_sb[:], wWT_d)
        nc.scalar.dma_start(bt_sb[:], bt_d)
        nc.scalar.dma_start(bWp_sb[:], bWp_d)
        x_dma(2)

        # ---- PE warmup: keep the HAM activity window busy while the x DMA
        # streams in, so real matmuls start at 2.4 GHz instead of 1.2.
        warm_ps = ps.tile([P, NQ], F32, tag="s", name="warm_ps")
        for _ in range(WARMUP_MMS):
            nc.tensor.matmul(warm_ps[:, 0:P], ones_sb[:], ones_sb[:],
                             start=True, stop=True, skip_group_check=True)

        th_sb = big.tile([P, N], FP16, name="th_sb")   # theta^T (i, n)
        ph_sb = big.tile([P, N], FP16, name="ph_sb")   # phi (i, m)
        gT_sb = big.tile([P, N], BF16, name="gT_sb")   # g0^T (o, m)
        g_sb = big.tile([P, MC, P], BF16, name="g_sb")  # g0 (m_in, chunk, o)

        def proj_block(b, wT, dst, bias):
            sl = slice(b * 512, (b + 1) * 512)
            p_ps = ps.tile([P, NQ], F32, tag="s", name="p_ps")
            nc.tensor.matmul(p_ps[:, 0:512], wT[:, 0], x_sb[:, 0, sl],
                             start=True, stop=False)
            nc.tensor.matmul(p_ps[:, 0:512], wT[:, 1], x_sb[:, 1, sl],
                             start=False, stop=True)
            if bias is None:
                nc.scalar.copy(dst[:, sl], p_ps[:, 0:512])
            else:
                nc.scalar.activation(dst[:, sl], p_ps[:, 0:512], AF.Identity,
                                     bias=bias)

        # per-quarter state carried across the flat loop
        state = {}

        def start_quarter(q):
            state[q] = {
                "y": psy.tile([P, NQ], F32, tag="y", name=f"y{q}_ps"),
                "acc": [None, None],
                "exp": [None] * MC,
            }

        def emit_S(q, mc):
            st = state[q]
            msl = slice(mc * P, (mc + 1) * P)
            s_ps = ps.tile([P, NQ], F32, tag="s", name="s_ps")
            for b in range(NB):
                nc.tensor.matmul(
                    s_ps[:, b * 512:(b + 1) * 512], ph_sb[:, msl],
                    th_sb[:, q * NQ + b * 512: q * NQ + (b + 1) * 512],
                    start=True, stop=True)
            exp_t = work.tile([P, NQ], BF16, tag="exp", bufs=7, name="exp_sb")
            if mc in DVE_EXP_MCS:
                nc.vector.tensor_scalar(
                    exp_t[:].bitcast(U16), s_ps[:], SCHR_MUL, SCHR_ADD,
                    ALU.mult, ALU.add)
            else:
                nc.scalar.activation(exp_t[:], s_ps[:], AF.Exp,
                                     bias=cshift_sb[:, 0:1])
            st["exp"][mc] = exp_t

        def emit_AV(q, mc):
            st = state[q]
            exp_t = st["exp"][mc]
            for b in range(NB):
                bsl = slice(b * 512, (b + 1) * 512)
                nc.tensor.matmul(
                    st["y"][:, bsl], g_sb[:, mc], exp_t[:, bsl],
                    start=(mc == 0), stop=(mc == MC - 1),
                    skip_group_check=True)
            j = mc % 2
            if st["acc"][j] is None:
                st["acc"][j] = work.tile([P, NQ], BF16, tag=f"acc{j}",
                                         bufs=2, name=f"acc{j}_sb")
                nc.vector.tensor_copy(st["acc"][j][:], exp_t[:])
            else:
                nc.vector.tensor_add(st["acc"][j][:], st["acc"][j][:],
                                     exp_t[:])
            st["exp"][mc] = None

        def finish_quarter(q):
            # flush the AV skew, then fold the denominator partials
            emit_AV(q, MC - 3)
            emit_AV(q, MC - 2)
            emit_AV(q, MC - 1)
            st = state[q]
            sumt = ps.tile([P, NQ], F32, tag="s", name="sumt_ps")
            for b in range(NB):
                bsl = slice(b * 512, (b + 1) * 512)
                for j in range(2):
                    nc.tensor.matmul(sumt[:, bsl], ones_sb[:],
                                     st["acc"][j][:, bsl],
                                     start=(j == 0), stop=(j == 1),
                                     skip_group_check=True)
            st["sumt"] = sumt

        def emit_norm(q):
            """1/sums and normalized y^T, both 512-col blocks (DVE)."""
            st = state[q]
            st["recip"] = work.tile([P, NQ], F32, tag="recip", bufs=2,
                                    name="recip_sb")
            st["yt"] = work.tile([P, NQ], F32R, tag="yt", bufs=2,
                                 name="yt_sb")
            for b in range(NB):
                bsl = slice(b * 512, (b + 1) * 512)
                nc.vector.reciprocal_approx_fast(st["recip"][:, bsl],
                                                 st["sumt"][:, bsl])
                nc.vector.tensor_mul(st["yt"][:, bsl], st["y"][:, bsl],
                                     st["recip"][:, bsl])

        def emit_wy(q):
            st = state[q]
            st["wy"] = [ps.tile([P, NQ], F32, tag="s", name=f"wy{h}_ps")
                        for h in range(2)]
            for b in range(NB):
                bsl = slice(b * 512, (b + 1) * 512)
                for h in range(2):
                    nc.tensor.matmul(st["wy"][h][:, bsl],
                                     wWT_sb[:, h * P:(h + 1) * P],
                                     st["yt"][:, bsl], start=True, stop=True)

        def emit_o(q, add_engine):
            st = state[q]
            o = [work.tile([P, NQ], F32, tag=f"o{h}", bufs=2,
                           name=f"o{h}_sb") for h in range(2)]
            for b in range(NB):
                bsl = slice(b * 512, (b + 1) * 512)
                csl = slice(q * NQ + b * 512, q * NQ + (b + 1) * 512)
                for h in range(2):
                    nc.scalar.activation(o[h][:, bsl], st["wy"][h][:, bsl],
                                         AF.Identity, bias=bWp_sb[:, h:h + 1])
                    add_engine.tensor_add(o[h][:, bsl], o[h][:, bsl],
                                          x_sb[:, h, csl])
                    nc.sync.dma_start(out_v[h, :, csl], o[h][:, bsl])

        # ---- emission ----
        th_ready = 0

        def emit_th(b):
            proj_block(b, wtT_sb, th_sb, bt_sb[:, 0:1])

        emit_th(0)
        emit_th(1)

        for t in range(NQn * MC):
            q, mc = divmod(t, MC)
            if mc == 0:
                start_quarter(q)
            if q == 0 and mc % 4 == 0:
                b = mc // 4
                if b + 3 <= 7:
                    x_dma(b + 3)
                proj_block(b, wpT_sb, ph_sb, None)     # phi keys block
                if G_VIA_TRANSPOSE:
                    proj_block(b, wgT_sb, gT_sb, None)  # gT keys block
                    for m2 in range(4 * b, 4 * b + 4):  # g chunks via xbar
                        nc.sync.dma_start_transpose(
                            g_sb[:, m2], gT_sb[:, m2 * P:(m2 + 1) * P])
                else:
                    for m2 in range(4 * b, 4 * b + 4):
                        g_ps = ps.tile([P, NQ], F32, tag="s", name="g_ps")
                        msl = slice(m2 * P, (m2 + 1) * P)
                        nc.tensor.matmul(g_ps[:, 0:P], x_sb[:, 0, msl],
                                         wgT_sb[:, 0], start=True, stop=False)
                        nc.tensor.matmul(g_ps[:, 0:P], x_sb[:, 1, msl],
                                         wgT_sb[:, 1], start=False, stop=True)
                        if m2 % 2 == 0:
                            nc.vector.tensor_copy(g_sb[:, m2], g_ps[:, 0:P])
                        else:
                            nc.scalar.copy(g_sb[:, m2], g_ps[:, 0:P])
            if q < NQn - 1 and mc in (8, 16):
                emit_th(2 * (q + 1) + (mc == 16))
            # previous quarter's pipelined tail, emitted BEFORE this step's
            # psum allocations so pool recycling can't outrun its readers
            if q > 0:
                if mc == 2:
                    emit_norm(q - 1)
                elif mc == 3:
                    emit_wy(q - 1)
                elif mc == 4:
                    emit_o(q - 1, nc.vector)
                    del state[q - 1]
            emit_S(q, mc)
            if q > 0 and mc == 1:
                finish_quarter(q - 1)
            # AV skew: 3 behind normally; first AVs of q>0 wait for the
            # previous quarter's y_ps to be fully consumed (psy bufs=1)
            if q == 0:
                if mc >= 3:
                    emit_AV(q, mc - 3)
            else:
                if mc in (3, 4, 5):
                    emit_AV(q, mc - 3)
                elif mc >= 6:
                    emit_AV(q, mc - 3)

        # final quarter: block-pipelined tail so the first 512 columns hit
        # the output DMA while the second block is still normalizing
        qf = NQn - 1
        finish_quarter(qf)
        st = state[qf]
        st["recip"] = work.tile([P, NQ], F32, tag="recip", bufs=2,
                                name="recip_sb")
        st["yt"] = work.tile([P, NQ], F32R, tag="yt", bufs=2, name="yt_sb")
        st["wy"] = [ps.tile([P, NQ], F32, tag="s", name=f"wyf{h}_ps")
                    for h in range(2)]
        of = [work.tile([P, NQ], F32, tag=f"o{h}", bufs=2, name=f"of{h}_sb")
              for h in range(2)]
        for b in range(NB):
            bsl = slice(b * 512, (b + 1) * 512)
            csl = slice(qf * NQ + b * 512, qf * NQ + (b + 1) * 512)
            nc.vector.reciprocal_approx_fast(st["recip"][:, bsl],
                                             st["sumt"][:, bsl])
            nc.vector.tensor_mul(st["yt"][:, bsl], st["y"][:, bsl],
                                 st["recip"][:, bsl])
            for h in range(2):
                nc.tensor.matmul(st["wy"][h][:, bsl],
                                 wWT_sb[:, h * P:(h + 1) * P],
                                 st["yt"][:, bsl], start=True, stop=True)
            for h in range(2):
                nc.scalar.activation(of[h][:, bsl], st["wy"][h][:, bsl],
                                     AF.Identity, bias=bWp_sb[:, h:h + 1])
                nc.vector.tensor_add(of[h][:, bsl], of[h][:, bsl],
                                     x_sb[:, h, csl])
                nc.sync.dma_start(out_v[h, :, csl], of[h][:, bsl])

    nc.compile()
    return nc


_CACHE = {}


def _built(key=(N_FULL,)):
    if key not in _CACHE:
        _CACHE[key] = build_nc(*key)
    return _CACHE[key]


def make_in_maps(x, wg, bg, wt, bt, wp, bp, wW, bW):
    """Host-side prep: per-core input dicts (core b <- batch b)."""
    x = np.asarray(x, np.float32)
    B, C_, H, W = x.shape
    N = H * W
    xf = np.ascontiguousarray(x.reshape(B, C_, N))
    wg, bg, wt, bt, wp, bp, wW, bW = [
        np.asarray(a, np.float32) for a in (wg, bg, wt, bt, wp, bp, wW, bW)]

    def pack(w):  # (128, C) conv weight -> partition-major lhsT chunks
        return np.ascontiguousarray(
            w.T.reshape(2, P, P).transpose(1, 0, 2).reshape(P, 2 * P))

    wtT, wpT, wgT = pack(wt), pack(wp), pack(wg)
    wWT = np.ascontiguousarray(wW.T)                       # (128, 256)
    bWp = (wW @ bg + bW).astype(np.float32)                # fold bg into bW
    bWp = np.ascontiguousarray(bWp.reshape(2, P).T)        # (128, 2)
    shared = {
        "wtT": wtT, "wpT": wpT, "wgT": wgT, "wWT": wWT,
        "bt": bt.reshape(P, 1).copy(), "bWp": bWp,
    }
    return [{"x": np.ascontiguousarray(xf[b]), **shared} for b in range(B)]


def kernel(x, wg, bg, wt, bt, wp, bp, wW, bW):
    from concourse.bass_utils import run_bass_kernel_spmd

    B, C_, H, W = np.asarray(x).shape
    in_maps = make_in_maps(x, wg, bg, wt, bt, wp, bp, wW, bW)
    nc = _built()
    res = run_bass_kernel_spmd(nc, in_maps, core_ids=list(range(B)))
    out = np.stack([res.results[b]["out"] for b in range(B)])
    return out.reshape(B, C_, H, W).astype(np.float32)
